# revision 10
# baseline (speedup 1.0000x reference)
"""Trainium2 Bass kernel for nn_MultiHeadAttention_8684423872640.

Math: the reference collapses algebraically. With
  s[m]   = Wfc[0, m // 64] / sqrt(64)
  Abar   = (Wk * s[:,None]).T @ Wq / L          # [1024, 1024] weights-only
  u      = Wk.T @ (s * bq)                      # [1024]
  qv     = Wq.T @ (s * bk) / L                  # [1024]
  c0     = (s * bk) @ bq + bfc[0]
the output for batch b is
  xsum_b = sum_l x[b, l, :]                     # [1024]
  w_eff  = Abar @ xsum_b + u                    # [1024]
  c      = qv @ xsum_b + c0
  out[b, l, 0] = x[b, l, :] @ w_eff + c

Sharding: data-parallel over B — core c handles batch c. Each core:
  pass 1: DMA x[b].T tiles [128, 4096] to SBUF, VectorE row-sums -> xsum
          (incrementally per 128-feature tile), TensorE folds each xsum
          p-tile into w_eff/c via Abar-block matmuls as soon as it's ready
  pass 2: TensorE matvec out[l] = xT[:, l] . w_eff (w_eff stationary,
          x streams as moving operand), +c epilogue on VectorE, DMA out.
"""

import os
import sys
import functools
import numpy as np

B, L, N = 8, 4096, 1024
D_K = 64
NCORES = 8
PT = N // 128  # 8 feature tiles
LCH = 512      # pass-2 moving chunk (fp32 max)
NLC = L // LCH

_TRN_REPO = "/opt/trn_rl_repo"


def _ensure_path():
    if _TRN_REPO not in sys.path and os.path.isdir(_TRN_REPO):
        sys.path.insert(0, _TRN_REPO)


@functools.lru_cache(maxsize=2)
def _build(x_dt_name: str = "float32", tail_split: int = 4, warmup_mms: int = 16):
    """Build + compile the per-core Bass program. Returns the finalized nc."""
    _ensure_path()
    import concourse.bass as bass
    import concourse.tile as tile
    from concourse import bacc, mybir

    f32 = mybir.dt.float32
    dtx = getattr(mybir.dt, x_dt_name)

    nc = bacc.Bacc(
        "TRN2",
        target_bir_lowering=False,
        debug=False,
        enable_asserts=False,
        num_devices=NCORES,
    )

    xT = nc.dram_tensor("xT", [N, L], dtx, kind="ExternalInput").ap()
    atr = nc.dram_tensor("atr", [128, PT * N], dtx, kind="ExternalInput").ap()
    qv8 = nc.dram_tensor("qv8", [128, PT], dtx, kind="ExternalInput").ap()
    u8 = nc.dram_tensor("u8", [128, PT], f32, kind="ExternalInput").ap()
    c0 = nc.dram_tensor("c0", [1, 1], f32, kind="ExternalInput").ap()
    out_d = nc.dram_tensor("out", [1, L], f32, kind="ExternalOutput").ap()

    with tile.TileContext(nc) as tc:
        with (
            tc.tile_pool(name="xpool", bufs=PT) as xpool,
            tc.tile_pool(name="cpool", bufs=1) as cpool,
            tc.tile_pool(name="spool", bufs=2) as spool,
            tc.tile_pool(name="xsums", bufs=PT + 2) as xsums,
            tc.tile_pool(name="wps", bufs=2, space="PSUM") as wps,
            tc.tile_pool(name="cps", bufs=1, space="PSUM") as cps,
            tc.tile_pool(name="ops", bufs=2, space="PSUM") as ops,
            tc.tile_pool(name="wrm", bufs=1, space="PSUM") as wrm,
        ):
            # -- constant / weight loads (front of DMA queue) --
            at_sb = cpool.tile([128, PT * N], dtx, tag="at")
            nc.sync.dma_start(at_sb[:], atr[:])
            qv_sb = cpool.tile([128, PT], dtx, tag="qv")
            nc.sync.dma_start(qv_sb[:], qv8[:])
            u_sb = cpool.tile([128, PT], f32, tag="u")
            nc.sync.dma_start(u_sb[:], u8[:])
            c0_sb = cpool.tile([1, 1], f32, tag="c0")
            nc.sync.dma_start(c0_sb[:], c0[:])

            # -- x tiles: big DMAs; last tile split for lower tail latency --
            x_sb = []
            for pt in range(PT):
                t = xpool.tile([128, L], dtx, tag="x")
                x_sb.append(t)
                if pt < PT - 1 or tail_split <= 1:
                    nc.sync.dma_start(t[:], xT[pt * 128:(pt + 1) * 128, :])
                else:
                    step = L // tail_split
                    for j in range(tail_split):
                        nc.sync.dma_start(
                            t[:, j * step:(j + 1) * step],
                            xT[pt * 128:(pt + 1) * 128, j * step:(j + 1) * step],
                        )

            c_ps = cps.tile([1, 1], f32, tag="cps")
            w8_acc = spool.tile([128, PT], f32, tag="w8acc")

            def to_mm_dtype(xs):
                """MM operands must match at_sb's dtype."""
                if dtx == f32:
                    return xs
                xm = xsums.tile([128, 1], dtx, tag="xsmm")
                nc.vector.tensor_copy(xm[:], xs[:])
                return xm

            def fold_ptile(pt, xs):
                """Add Abar-block @ xsum_pt into w8_acc / c_ps."""
                xm = to_mm_dtype(xs)
                wp = wps.tile([128, PT], f32, tag="wp")
                for nt in range(PT):
                    nc.tensor.matmul(
                        wp[:, nt:nt + 1],
                        at_sb[:, pt * N + nt * 128: pt * N + (nt + 1) * 128],
                        xm[:],
                        start=True, stop=True,
                    )
                nc.tensor.matmul(
                    c_ps[:], qv_sb[:, pt:pt + 1], xm[:],
                    start=(pt == 0), stop=(pt == PT - 1),
                )
                if pt == 0:
                    nc.vector.tensor_copy(w8_acc[:], wp[:])
                else:
                    nc.vector.tensor_add(w8_acc[:], w8_acc[:], wp[:])

            # -- pass 1: reduce + incremental fold --
            for pt in range(PT - 1):
                xs = xsums.tile([128, 1], f32, tag="xsum")
                nc.vector.tensor_reduce(
                    xs[:], x_sb[pt][:], axis=mybir.AxisListType.X,
                    op=mybir.AluOpType.add,
                )
                fold_ptile(pt, xs)

            # PE warm-up: junk matmuls to lift HAM to 8/8 before the tail.
            if warmup_mms:
                wscr = wrm.tile([1, LCH], f32, tag="warm")
                for i in range(warmup_mms):
                    nc.tensor.matmul(
                        wscr[:], qv_sb[:, 0:1], x_sb[0][:, 0:LCH],
                        start=(i == 0), stop=(i == warmup_mms - 1),
                    )

            # tail tile: chunked reduce to shorten the critical path
            pt = PT - 1
            if tail_split > 1:
                step = L // tail_split
                parts = xsums.tile([128, tail_split], f32, tag="parts")
                for j in range(tail_split):
                    nc.vector.tensor_reduce(
                        parts[:, j:j + 1], x_sb[pt][:, j * step:(j + 1) * step],
                        axis=mybir.AxisListType.X, op=mybir.AluOpType.add,
                    )
                xs = xsums.tile([128, 1], f32, tag="xsum")
                nc.vector.tensor_reduce(
                    xs[:], parts[:], axis=mybir.AxisListType.X,
                    op=mybir.AluOpType.add,
                )
            else:
                xs = xsums.tile([128, 1], f32, tag="xsum")
                nc.vector.tensor_reduce(
                    xs[:], x_sb[pt][:], axis=mybir.AxisListType.X,
                    op=mybir.AluOpType.add,
                )
            fold_ptile(pt, xs)

            # -- finalize w_eff / c --
            w_eff = spool.tile([128, PT], dtx, tag="weff")
            nc.vector.tensor_add(w_eff[:], w8_acc[:], u_sb[:])
            c_sb = spool.tile([1, 1], f32, tag="csb")
            nc.vector.tensor_add(c_sb[:], c_ps[:], c0_sb[:])

            # -- pass 2: out[l] = xT[:, l] . w_eff + c --
            out_sb = cpool.tile([1, L], f32, tag="osb")
            for lc in range(NLC):
                o_ps = ops.tile([1, LCH], f32, tag="ops")
                for nt in range(PT):
                    nc.tensor.matmul(
                        o_ps[:],
                        w_eff[:, nt:nt + 1],
                        x_sb[nt][:, lc * LCH:(lc + 1) * LCH],
                        start=(nt == 0), stop=(nt == PT - 1),
                    )
                nc.vector.tensor_scalar_add(
                    out_sb[0:1, lc * LCH:(lc + 1) * LCH], o_ps[:], c_sb[0:1, 0:1],
                )
            nc.sync.dma_start(out_d[:], out_sb[:])

    nc.compile()
    return nc


def _prep_host(inputs, x_dt_name="float32"):
    """Fold weights on host (f64 accumulate) and lay out per-core arrays."""
    Wq = np.asarray(inputs["Wq"], np.float64)
    bq = np.asarray(inputs["bq"], np.float64)
    Wk = np.asarray(inputs["Wk"], np.float64)
    bk = np.asarray(inputs["bk"], np.float64)
    Wfc = np.asarray(inputs["Wfc"], np.float64)
    bfc = np.asarray(inputs["bfc"], np.float64)

    s = np.repeat(Wfc[0], D_K) / np.sqrt(D_K)
    A = (Wk * s[:, None]).T @ Wq / L          # [n, p]
    u = Wk.T @ (s * bq)                       # [n]
    qv = Wq.T @ (s * bk) / L                  # [p]
    c0 = float((s * bk) @ bq + bfc[0])

    np_dtx = {"float32": np.float32, "bfloat16": None}[x_dt_name]
    if np_dtx is None:
        import ml_dtypes
        np_dtx = ml_dtypes.bfloat16

    at = np.ascontiguousarray(A.T)            # [p, n]
    atr = np.ascontiguousarray(
        at.reshape(PT, 128, N).transpose(1, 0, 2).reshape(128, PT * N)
    ).astype(np_dtx)
    qv8 = np.ascontiguousarray(qv.reshape(PT, 128).T).astype(np_dtx)
    u8 = np.ascontiguousarray(u.reshape(PT, 128).T).astype(np.float32)
    c0a = np.full((1, 1), c0, np.float32)

    x = np.asarray(inputs["x"])
    shared = {"atr": atr, "qv8": qv8, "u8": u8, "c0": c0a}
    in_maps = []
    for c in range(NCORES):
        m = dict(shared)
        m["xT"] = np.ascontiguousarray(x[c].T).astype(np_dtx, copy=False)
        in_maps.append(m)
    return in_maps


_X_DT = os.environ.get("KERNEL_X_DT", "float32")
LAST_RESULTS = None


def kernel(**inputs) -> np.ndarray:
    global LAST_RESULTS
    _ensure_path()
    from concourse.bass_utils import run_bass_kernel_spmd

    nc = _build(_X_DT)
    in_maps = _prep_host(inputs, _X_DT)
    kw = {}
    if os.environ.get("KERNEL_TRACE"):
        kw["trace"] = True
    res = run_bass_kernel_spmd(nc, in_maps, list(range(NCORES)), **kw)
    LAST_RESULTS = res
    out = np.stack([res.results[c]["out"].reshape(L, 1) for c in range(NCORES)])
    return out.astype(np.float32)


if __name__ == "__main__":
    rng = np.random.default_rng(0)
    demo = {
        "x": rng.standard_normal((B, L, N), np.float32),
        "Wq": rng.standard_normal((N, N), np.float32) * 0.03,
        "bq": rng.standard_normal((N,), np.float32) * 0.03,
        "Wk": rng.standard_normal((N, N), np.float32) * 0.03,
        "bk": rng.standard_normal((N,), np.float32) * 0.03,
        "Wfc": rng.standard_normal((1, 16), np.float32) * 0.25,
        "bfc": rng.standard_normal((1,), np.float32) * 0.25,
    }
    o = kernel(**demo)
    print("out", o.shape, o.dtype, float(np.abs(o).max()))


# revision 13
# speedup vs baseline: 1.8841x; 1.8841x over previous
"""Trainium2 Bass kernel for nn_MultiHeadAttention_8684423872640.

Math: the reference collapses algebraically. With
  s[m]   = Wfc[0, m // 64] / sqrt(64)
  Abar   = (Wk * s[:,None]).T @ Wq / L          # [1024, 1024] weights-only
  u      = Wk.T @ (s * bq)                      # [1024]
  qv     = Wq.T @ (s * bk) / L                  # [1024]
  c0     = (s * bk) @ bq + bfc[0]
the output for batch b is
  xsum_b = sum_l x[b, l, :]                     # [1024]
  w_eff  = Abar @ xsum_b + u                    # [1024]
  c      = qv @ xsum_b + c0
  out[b, l, 0] = x[b, l, :] @ w_eff + c

Sharding: data-parallel over B — core c handles batch c. Each core:
  pass 1: DMA x[b].T tiles [128, 4096] to SBUF, VectorE row-sums -> xsum
          (incrementally per 128-feature tile), TensorE folds each xsum
          p-tile into w_eff/c via Abar-block matmuls as soon as it's ready
  pass 2: TensorE matvec out[l] = xT[:, l] . w_eff (w_eff stationary,
          x streams as moving operand), +c epilogue on VectorE, DMA out.
"""

import os
import sys
import functools
import numpy as np

B, L, N = 8, 4096, 1024
D_K = 64
NCORES = 8
PT = N // 128  # 8 feature tiles
LCH = 512      # pass-2 moving chunk (fp32 max)
NLC = L // LCH

_TRN_REPO = "/opt/trn_rl_repo"


def _ensure_path():
    if _TRN_REPO not in sys.path and os.path.isdir(_TRN_REPO):
        sys.path.insert(0, _TRN_REPO)


@functools.lru_cache(maxsize=2)
def _build(x_dt_name: str = "float32", tail_split: int = 4, warmup_mms: int = 0):
    """Build + compile the per-core Bass program. Returns the finalized nc."""
    _ensure_path()
    import concourse.bass as bass
    import concourse.tile as tile
    from concourse import bacc, mybir

    f32 = mybir.dt.float32
    dtx = getattr(mybir.dt, x_dt_name)

    nc = bacc.Bacc(
        "TRN2",
        target_bir_lowering=False,
        debug=False,
        enable_asserts=False,
        num_devices=NCORES,
    )

    xT = nc.dram_tensor("xT", [N, L], dtx, kind="ExternalInput").ap()
    atr = nc.dram_tensor("atr", [128, PT * N], dtx, kind="ExternalInput").ap()
    qv8 = nc.dram_tensor("qv8", [128, PT], dtx, kind="ExternalInput").ap()
    u8 = nc.dram_tensor("u8", [128, PT], f32, kind="ExternalInput").ap()
    c0 = nc.dram_tensor("c0", [1, 1], f32, kind="ExternalInput").ap()
    out_d = nc.dram_tensor("out", [1, L], f32, kind="ExternalOutput").ap()

    with tile.TileContext(nc) as tc:
        with (
            tc.tile_pool(name="xpool", bufs=PT) as xpool,
            tc.tile_pool(name="cpool", bufs=1) as cpool,
            tc.tile_pool(name="spool", bufs=2) as spool,
            tc.tile_pool(name="xsums", bufs=PT + 2) as xsums,
            tc.tile_pool(name="wps", bufs=2, space="PSUM") as wps,
            tc.tile_pool(name="cps", bufs=1, space="PSUM") as cps,
            tc.tile_pool(name="ops", bufs=2, space="PSUM") as ops,
            tc.tile_pool(name="wrm", bufs=1, space="PSUM") as wrm,
        ):
            # Two HWDGE rings (SP + ACT): alternate big transfers so one
            # ring's ~2us completion receipt hides under the other's data.
            rings = [nc.sync, nc.scalar]

            # -- constant / weight loads (front of DMA queue) --
            at_sb = cpool.tile([128, PT * N], dtx, tag="at")
            rings[1].dma_start(at_sb[:], atr[:])
            qv_sb = cpool.tile([128, PT], dtx, tag="qv")
            rings[1].dma_start(qv_sb[:], qv8[:])
            u_sb = cpool.tile([128, PT], f32, tag="u")
            rings[1].dma_start(u_sb[:], u8[:])
            c0_sb = cpool.tile([1, 1], f32, tag="c0")
            rings[1].dma_start(c0_sb[:], c0[:])

            # -- x tiles: big DMAs; last tile split for lower tail latency --
            x_sb = []
            for pt in range(PT):
                t = xpool.tile([128, L], dtx, tag="x")
                x_sb.append(t)
                if pt < PT - 1 or tail_split <= 1:
                    rings[pt % 2].dma_start(t[:], xT[pt * 128:(pt + 1) * 128, :])
                else:
                    step = L // tail_split
                    for j in range(tail_split):
                        rings[j % 2].dma_start(
                            t[:, j * step:(j + 1) * step],
                            xT[pt * 128:(pt + 1) * 128, j * step:(j + 1) * step],
                        )

            c_ps = cps.tile([1, 1], f32, tag="cps")
            w8_acc = spool.tile([128, PT], f32, tag="w8acc")

            def to_mm_dtype(xs):
                """MM operands must match at_sb's dtype."""
                if dtx == f32:
                    return xs
                xm = xsums.tile([128, 1], dtx, tag="xsmm")
                nc.vector.tensor_copy(xm[:], xs[:])
                return xm

            def fold_ptile(pt, xs):
                """Add Abar-block @ xsum_pt into w8_acc / c_ps."""
                xm = to_mm_dtype(xs)
                wp = wps.tile([128, PT], f32, tag="wp")
                for nt in range(PT):
                    nc.tensor.matmul(
                        wp[:, nt:nt + 1],
                        at_sb[:, pt * N + nt * 128: pt * N + (nt + 1) * 128],
                        xm[:],
                        start=True, stop=True,
                    )
                nc.tensor.matmul(
                    c_ps[:], qv_sb[:, pt:pt + 1], xm[:],
                    start=(pt == 0), stop=(pt == PT - 1),
                )
                if pt == 0:
                    nc.vector.tensor_copy(w8_acc[:], wp[:])
                else:
                    nc.vector.tensor_add(w8_acc[:], w8_acc[:], wp[:])

            # -- pass 1: reduce + incremental fold --
            for pt in range(PT - 1):
                xs = xsums.tile([128, 1], f32, tag="xsum")
                nc.vector.tensor_reduce(
                    xs[:], x_sb[pt][:], axis=mybir.AxisListType.X,
                    op=mybir.AluOpType.add,
                )
                fold_ptile(pt, xs)

            # PE warm-up: junk matmuls to lift HAM to 8/8 before the tail.
            if warmup_mms:
                wscr = wrm.tile([1, LCH], f32, tag="warm")
                for i in range(warmup_mms):
                    nc.tensor.matmul(
                        wscr[:], qv_sb[:, 0:1], x_sb[0][:, 0:LCH],
                        start=(i == 0), stop=(i == warmup_mms - 1),
                    )

            # tail tile: chunked reduce to shorten the critical path
            pt = PT - 1
            if tail_split > 1:
                step = L // tail_split
                parts = xsums.tile([128, tail_split], f32, tag="parts")
                for j in range(tail_split):
                    nc.vector.tensor_reduce(
                        parts[:, j:j + 1], x_sb[pt][:, j * step:(j + 1) * step],
                        axis=mybir.AxisListType.X, op=mybir.AluOpType.add,
                    )
                xs = xsums.tile([128, 1], f32, tag="xsum")
                nc.vector.tensor_reduce(
                    xs[:], parts[:], axis=mybir.AxisListType.X,
                    op=mybir.AluOpType.add,
                )
            else:
                xs = xsums.tile([128, 1], f32, tag="xsum")
                nc.vector.tensor_reduce(
                    xs[:], x_sb[pt][:], axis=mybir.AxisListType.X,
                    op=mybir.AluOpType.add,
                )
            fold_ptile(pt, xs)

            # -- finalize w_eff / c --
            w_eff = spool.tile([128, PT], dtx, tag="weff")
            nc.vector.tensor_add(w_eff[:], w8_acc[:], u_sb[:])
            c_sb = spool.tile([1, 1], f32, tag="csb")
            nc.vector.tensor_add(c_sb[:], c_ps[:], c0_sb[:])

            # -- pass 2: out[l] = xT[:, l] . w_eff + c --
            out_sb = cpool.tile([1, L], f32, tag="osb")
            for lc in range(NLC):
                o_ps = ops.tile([1, LCH], f32, tag="ops")
                for nt in range(PT):
                    nc.tensor.matmul(
                        o_ps[:],
                        w_eff[:, nt:nt + 1],
                        x_sb[nt][:, lc * LCH:(lc + 1) * LCH],
                        start=(nt == 0), stop=(nt == PT - 1),
                    )
                nc.vector.tensor_scalar_add(
                    out_sb[0:1, lc * LCH:(lc + 1) * LCH], o_ps[:], c_sb[0:1, 0:1],
                )
            nc.sync.dma_start(out_d[:], out_sb[:])

    nc.compile()
    return nc


def _prep_host(inputs, x_dt_name="float32"):
    """Fold weights on host (f64 accumulate) and lay out per-core arrays."""
    Wq = np.asarray(inputs["Wq"], np.float64)
    bq = np.asarray(inputs["bq"], np.float64)
    Wk = np.asarray(inputs["Wk"], np.float64)
    bk = np.asarray(inputs["bk"], np.float64)
    Wfc = np.asarray(inputs["Wfc"], np.float64)
    bfc = np.asarray(inputs["bfc"], np.float64)

    s = np.repeat(Wfc[0], D_K) / np.sqrt(D_K)
    A = (Wk * s[:, None]).T @ Wq / L          # [n, p]
    u = Wk.T @ (s * bq)                       # [n]
    qv = Wq.T @ (s * bk) / L                  # [p]
    c0 = float((s * bk) @ bq + bfc[0])

    np_dtx = {"float32": np.float32, "bfloat16": None}[x_dt_name]
    if np_dtx is None:
        import ml_dtypes
        np_dtx = ml_dtypes.bfloat16

    at = np.ascontiguousarray(A.T)            # [p, n]
    atr = np.ascontiguousarray(
        at.reshape(PT, 128, N).transpose(1, 0, 2).reshape(128, PT * N)
    ).astype(np_dtx)
    qv8 = np.ascontiguousarray(qv.reshape(PT, 128).T).astype(np_dtx)
    u8 = np.ascontiguousarray(u.reshape(PT, 128).T).astype(np.float32)
    c0a = np.full((1, 1), c0, np.float32)

    x = np.asarray(inputs["x"])
    shared = {"atr": atr, "qv8": qv8, "u8": u8, "c0": c0a}
    in_maps = []
    for c in range(NCORES):
        m = dict(shared)
        m["xT"] = np.ascontiguousarray(x[c].T).astype(np_dtx, copy=False)
        in_maps.append(m)
    return in_maps


_X_DT = os.environ.get("KERNEL_X_DT", "bfloat16")
LAST_RESULTS = None


def kernel(**inputs) -> np.ndarray:
    global LAST_RESULTS
    _ensure_path()
    from concourse.bass_utils import run_bass_kernel_spmd

    nc = _build(_X_DT)
    in_maps = _prep_host(inputs, _X_DT)
    kw = {}
    if os.environ.get("KERNEL_TRACE"):
        kw["trace"] = True
    res = run_bass_kernel_spmd(nc, in_maps, list(range(NCORES)), **kw)
    LAST_RESULTS = res
    out = np.stack([res.results[c]["out"].reshape(L, 1) for c in range(NCORES)])
    return out.astype(np.float32)


if __name__ == "__main__":
    rng = np.random.default_rng(0)
    demo = {
        "x": rng.standard_normal((B, L, N), np.float32),
        "Wq": rng.standard_normal((N, N), np.float32) * 0.03,
        "bq": rng.standard_normal((N,), np.float32) * 0.03,
        "Wk": rng.standard_normal((N, N), np.float32) * 0.03,
        "bk": rng.standard_normal((N,), np.float32) * 0.03,
        "Wfc": rng.standard_normal((1, 16), np.float32) * 0.25,
        "bfc": rng.standard_normal((1,), np.float32) * 0.25,
    }
    o = kernel(**demo)
    print("out", o.shape, o.dtype, float(np.abs(o).max()))


# revision 16
# speedup vs baseline: 2.1519x; 1.1421x over previous
"""Trainium2 Bass kernel for nn_MultiHeadAttention_8684423872640.

Math: the reference collapses algebraically. With
  s[m]   = Wfc[0, m // 64] / sqrt(64)
  Abar   = (Wk * s[:,None]).T @ Wq / L          # [1024, 1024] weights-only
  u      = Wk.T @ (s * bq)                      # [1024]
  qv     = Wq.T @ (s * bk) / L                  # [1024]
  c0     = (s * bk) @ bq + bfc[0]
the output for batch b is
  xsum_b = sum_l x[b, l, :]                     # [1024]
  w_eff  = Abar @ xsum_b + u                    # [1024]
  c      = qv @ xsum_b + c0
  out[b, l, 0] = x[b, l, :] @ w_eff + c

Sharding: data-parallel over B — core c handles batch c. Each core:
  pass 1: DMA x[b].T tiles [128, 4096] to SBUF, VectorE row-sums -> xsum
          (incrementally per 128-feature tile), TensorE folds each xsum
          p-tile into w_eff/c via Abar-block matmuls as soon as it's ready
  pass 2: TensorE matvec out[l] = xT[:, l] . w_eff (w_eff stationary,
          x streams as moving operand), +c epilogue on VectorE, DMA out.
"""

import os
import sys
import functools
import numpy as np

B, L, N = 8, 4096, 1024
D_K = 64
NCORES = 8
PT = N // 128  # 8 feature tiles
LCH = 512      # pass-2 moving chunk (fp32 max)
NLC = L // LCH

_TRN_REPO = "/opt/trn_rl_repo"


def _ensure_path():
    if _TRN_REPO not in sys.path and os.path.isdir(_TRN_REPO):
        sys.path.insert(0, _TRN_REPO)


@functools.lru_cache(maxsize=2)
def _build(x_dt_name: str = "float32", tail_split: int = 4, warmup_mms: int = 0):
    """Build + compile the per-core Bass program. Returns the finalized nc."""
    _ensure_path()
    import concourse.bass as bass
    import concourse.tile as tile
    from concourse import bacc, mybir

    f32 = mybir.dt.float32
    dtx = getattr(mybir.dt, x_dt_name)

    nc = bacc.Bacc(
        "TRN2",
        target_bir_lowering=False,
        debug=False,
        enable_asserts=False,
        num_devices=NCORES,
    )

    xT = nc.dram_tensor("xT", [N, L], dtx, kind="ExternalInput").ap()
    atr = nc.dram_tensor("atr", [128, PT * N], dtx, kind="ExternalInput").ap()
    qv8 = nc.dram_tensor("qv8", [128, PT], dtx, kind="ExternalInput").ap()
    u8 = nc.dram_tensor("u8", [128, PT], f32, kind="ExternalInput").ap()
    c0 = nc.dram_tensor("c0", [1, 1], f32, kind="ExternalInput").ap()
    out_d = nc.dram_tensor("out", [1, L], f32, kind="ExternalOutput").ap()

    with tile.TileContext(nc) as tc:
        with (
            tc.tile_pool(name="xpool", bufs=PT) as xpool,
            tc.tile_pool(name="cpool", bufs=1) as cpool,
            tc.tile_pool(name="spool", bufs=2) as spool,
            tc.tile_pool(name="xsums", bufs=PT + 2) as xsums,
            tc.tile_pool(name="wps", bufs=2, space="PSUM") as wps,
            tc.tile_pool(name="cps", bufs=1, space="PSUM") as cps,
            tc.tile_pool(name="ops", bufs=3, space="PSUM") as ops,
            tc.tile_pool(name="wrm", bufs=1, space="PSUM") as wrm,
        ):
            # Two HWDGE rings (SP + ACT): alternate big transfers so one
            # ring's ~2us completion receipt hides under the other's data.
            rings = [nc.sync, nc.scalar]

            # -- constant / weight loads (front of DMA queue) --
            at_sb = cpool.tile([128, PT * N], dtx, tag="at")
            rings[1].dma_start(at_sb[:], atr[:])
            qv_sb = cpool.tile([128, PT], dtx, tag="qv")
            rings[1].dma_start(qv_sb[:], qv8[:])
            u_sb = cpool.tile([128, PT], f32, tag="u")
            rings[1].dma_start(u_sb[:], u8[:])
            c0_sb = cpool.tile([1, 1], f32, tag="c0")
            rings[1].dma_start(c0_sb[:], c0[:])

            # -- x tiles: big DMAs; last tile split for lower tail latency --
            x_sb = []
            for pt in range(PT):
                t = xpool.tile([128, L], dtx, tag="x")
                x_sb.append(t)
                if pt < PT - 1 or tail_split <= 1:
                    rings[pt % 2].dma_start(t[:], xT[pt * 128:(pt + 1) * 128, :])
                else:
                    step = L // tail_split
                    for j in range(tail_split):
                        rings[j % 2].dma_start(
                            t[:, j * step:(j + 1) * step],
                            xT[pt * 128:(pt + 1) * 128, j * step:(j + 1) * step],
                        )

            c_ps = cps.tile([1, 1], f32, tag="cps")
            w8_acc = spool.tile([128, PT], f32, tag="w8acc")

            def to_mm_dtype(xs):
                """MM operands must match at_sb's dtype."""
                if dtx == f32:
                    return xs
                xm = xsums.tile([128, 1], dtx, tag="xsmm")
                nc.vector.tensor_copy(xm[:], xs[:])
                return xm

            def fold_ptile(pt, xs):
                """Add Abar-block @ xsum_pt into w8_acc / c_ps."""
                xm = to_mm_dtype(xs)
                wp = wps.tile([128, PT], f32, tag="wp")
                for nt in range(PT):
                    nc.tensor.matmul(
                        wp[:, nt:nt + 1],
                        at_sb[:, pt * N + nt * 128: pt * N + (nt + 1) * 128],
                        xm[:],
                        start=True, stop=True,
                    )
                nc.tensor.matmul(
                    c_ps[:], qv_sb[:, pt:pt + 1], xm[:],
                    start=(pt == 0), stop=(pt == PT - 1),
                )
                if pt == 0:
                    nc.vector.tensor_copy(w8_acc[:], wp[:])
                else:
                    nc.vector.tensor_add(w8_acc[:], w8_acc[:], wp[:])

            # Row-sum engine split: tensor_reduce is a 1x-mode DVE op
            # (~4.4us/tile), so alternate tiles onto ScalarE via
            # activation(Copy, accum_out=...) to halve the reduction span.
            act_scr = cpool.tile([128, L], dtx, tag="ascr")

            def rowsum(src, xs_out, use_act):
                if use_act:
                    w = src.shape[-1]
                    nc.scalar.activation(
                        act_scr[:, 0:w], src, mybir.ActivationFunctionType.Copy,
                        bias=0.0, accum_out=xs_out,
                    )
                else:
                    nc.vector.tensor_reduce(
                        xs_out, src, axis=mybir.AxisListType.X,
                        op=mybir.AluOpType.add,
                    )

            # -- pass 1: reduce + incremental fold --
            for pt in range(PT - 1):
                xs = xsums.tile([128, 1], f32, tag="xsum")
                rowsum(x_sb[pt][:], xs[:], use_act=(pt % 2 == 0))
                fold_ptile(pt, xs)

            # PE warm-up: junk matmuls to lift HAM to 8/8 before the tail.
            if warmup_mms:
                wscr = wrm.tile([1, LCH], f32, tag="warm")
                for i in range(warmup_mms):
                    nc.tensor.matmul(
                        wscr[:], qv_sb[:, 0:1], x_sb[0][:, 0:LCH],
                        start=(i == 0), stop=(i == warmup_mms - 1),
                    )

            # tail tile: chunked reduce to shorten the critical path
            pt = PT - 1
            if tail_split > 1:
                step = L // tail_split
                parts = xsums.tile([128, tail_split], f32, tag="parts")
                for j in range(tail_split):
                    rowsum(x_sb[pt][:, j * step:(j + 1) * step],
                           parts[:, j:j + 1], use_act=(j % 2 == 0))
                xs = xsums.tile([128, 1], f32, tag="xsum")
                nc.vector.tensor_reduce(
                    xs[:], parts[:], axis=mybir.AxisListType.X,
                    op=mybir.AluOpType.add,
                )
            else:
                xs = xsums.tile([128, 1], f32, tag="xsum")
                rowsum(x_sb[pt][:], xs[:], use_act=False)
            fold_ptile(pt, xs)

            # -- finalize w_eff / c --
            w_eff = spool.tile([128, PT], dtx, tag="weff")
            nc.vector.tensor_add(w_eff[:], w8_acc[:], u_sb[:])
            c_sb = spool.tile([1, 1], f32, tag="csb")
            nc.vector.tensor_add(c_sb[:], c_ps[:], c0_sb[:])

            # -- pass 2: out[l] = xT[:, l] . w_eff + c --
            out_sb = cpool.tile([1, L], f32, tag="osb")
            for lc in range(NLC):
                o_ps = ops.tile([1, LCH], f32, tag="ops")
                for nt in range(PT):
                    nc.tensor.matmul(
                        o_ps[:],
                        w_eff[:, nt:nt + 1],
                        x_sb[nt][:, lc * LCH:(lc + 1) * LCH],
                        start=(nt == 0), stop=(nt == PT - 1),
                    )
                nc.vector.tensor_scalar_add(
                    out_sb[0:1, lc * LCH:(lc + 1) * LCH], o_ps[:], c_sb[0:1, 0:1],
                )
            nc.sync.dma_start(out_d[:], out_sb[:])

    nc.compile()
    return nc


def _prep_host(inputs, x_dt_name="float32"):
    """Fold weights on host (f64 accumulate) and lay out per-core arrays."""
    Wq = np.asarray(inputs["Wq"], np.float64)
    bq = np.asarray(inputs["bq"], np.float64)
    Wk = np.asarray(inputs["Wk"], np.float64)
    bk = np.asarray(inputs["bk"], np.float64)
    Wfc = np.asarray(inputs["Wfc"], np.float64)
    bfc = np.asarray(inputs["bfc"], np.float64)

    s = np.repeat(Wfc[0], D_K) / np.sqrt(D_K)
    A = (Wk * s[:, None]).T @ Wq / L          # [n, p]
    u = Wk.T @ (s * bq)                       # [n]
    qv = Wq.T @ (s * bk) / L                  # [p]
    c0 = float((s * bk) @ bq + bfc[0])

    np_dtx = {"float32": np.float32, "bfloat16": None}[x_dt_name]
    if np_dtx is None:
        import ml_dtypes
        np_dtx = ml_dtypes.bfloat16

    at = np.ascontiguousarray(A.T)            # [p, n]
    atr = np.ascontiguousarray(
        at.reshape(PT, 128, N).transpose(1, 0, 2).reshape(128, PT * N)
    ).astype(np_dtx)
    qv8 = np.ascontiguousarray(qv.reshape(PT, 128).T).astype(np_dtx)
    u8 = np.ascontiguousarray(u.reshape(PT, 128).T).astype(np.float32)
    c0a = np.full((1, 1), c0, np.float32)

    x = np.asarray(inputs["x"])
    shared = {"atr": atr, "qv8": qv8, "u8": u8, "c0": c0a}
    in_maps = []
    for c in range(NCORES):
        m = dict(shared)
        m["xT"] = np.ascontiguousarray(x[c].T).astype(np_dtx, copy=False)
        in_maps.append(m)
    return in_maps


_X_DT = os.environ.get("KERNEL_X_DT", "bfloat16")
LAST_RESULTS = None


def kernel(**inputs) -> np.ndarray:
    global LAST_RESULTS
    _ensure_path()
    from concourse.bass_utils import run_bass_kernel_spmd

    nc = _build(_X_DT)
    in_maps = _prep_host(inputs, _X_DT)
    kw = {}
    if os.environ.get("KERNEL_TRACE"):
        kw["trace"] = True
    res = run_bass_kernel_spmd(nc, in_maps, list(range(NCORES)), **kw)
    LAST_RESULTS = res
    out = np.stack([res.results[c]["out"].reshape(L, 1) for c in range(NCORES)])
    return out.astype(np.float32)


if __name__ == "__main__":
    rng = np.random.default_rng(0)
    demo = {
        "x": rng.standard_normal((B, L, N), np.float32),
        "Wq": rng.standard_normal((N, N), np.float32) * 0.03,
        "bq": rng.standard_normal((N,), np.float32) * 0.03,
        "Wk": rng.standard_normal((N, N), np.float32) * 0.03,
        "bk": rng.standard_normal((N,), np.float32) * 0.03,
        "Wfc": rng.standard_normal((1, 16), np.float32) * 0.25,
        "bfc": rng.standard_normal((1,), np.float32) * 0.25,
    }
    o = kernel(**demo)
    print("out", o.shape, o.dtype, float(np.abs(o).max()))
